# revision 14
# baseline (speedup 1.0000x reference)
"""AdaConv2D Trainium2 kernel: instance-norm + per-sample depthwise-separable
conv + dense 3x3 spatial conv + LeakyReLU, data-parallel over batch on 8 cores.

Per core (one batch sample):
  - z0[c] = sum_k dw[c,k] * x[c, window_k]   (raw depthwise, VectorE, f32 acc;
    starts immediately, independent of instance-norm stats)
  - instance norm + pointwise fold into the matmul side:
        out = sum_ci (W * s[ci])^T @ z0[ci] + c[co],  s = pw * rsqrt(var+eps)
        c[co] = sum_ci sum_k W[co,ci,k] * beta[ci],
        beta = bias - mean * s * sum_k dw
    so the dense 3x3 512->512 conv runs on TensorE as 36 accumulating bf16
    matmuls per (4-row band, output channel group) against scaled weights.
  - stats (mean, E[x^2]) stream on ScalarE (sum) + VectorE (sumsq) during the
    same window the depthwise warm-up runs, so TensorE starts ~100us in.
  - LeakyReLU fused as max(0.01*v, v) on VectorE; bias c+spatial_b on ScalarE.
Weights are host-packed/cast to bf16 in the lhsT layout the TensorE wants.
"""
import sys
import numpy as np

sys.path.insert(0, "/opt/trn_rl_repo")

import ml_dtypes  # noqa: E402

B, C, H, W = 8, 512, 128, 128
CG = C // 128          # 4 channel groups
PIX = H * W
BAND = 4               # output rows per matmul band
NBANDS = H // BAND
ZCH = 8                # z production chunk rows
NZCH = H // ZCH
RING = 40              # z ring rows (multiple of ZCH)
WPAD = W + 2           # width-padded row length
XCH = 2048             # pass-A stats chunk (free-dim cols)
NCH = PIX // XCH       # chunks per channel group
EPS = 1e-5
SLOPE = 0.01

_CACHE = {}


def _reflect(r):
    if r < 0:
        return -r
    if r > H - 1:
        return 2 * (H - 1) - r
    return r


def _build():
    from concourse import bacc, tile, mybir

    AF = mybir.ActivationFunctionType
    ALU = mybir.AluOpType
    F32 = mybir.dt.float32
    BF16 = mybir.dt.bfloat16

    nc = bacc.Bacc(None, target_bir_lowering=False, debug=False)

    x_ext = nc.declare_dram_parameter("x", [C, PIX], F32, isOutput=False)
    wt_ext = nc.declare_dram_parameter("wt", [128, CG * 9 * CG * 128], BF16, isOutput=False)
    dw_ext = nc.declare_dram_parameter("dw", [128, CG, 9], F32, isOutput=False)
    pw_ext = nc.declare_dram_parameter("pw", [128, CG], F32, isOutput=False)
    bias_ext = nc.declare_dram_parameter("bias", [128, CG], F32, isOutput=False)
    sb_ext = nc.declare_dram_parameter("sb", [128, CG], F32, isOutput=False)
    out_ext = nc.declare_dram_parameter("out", [C, PIX], F32, isOutput=True)

    with tile.TileContext(nc) as tc:
        with (
            tc.tile_pool(name="persist", bufs=1) as pp,
            tc.tile_pool(name="xa", bufs=3) as xa_pool,
            tc.tile_pool(name="scra", bufs=2) as scra_pool,
            tc.tile_pool(name="scrv", bufs=2) as scrv_pool,
            tc.tile_pool(name="xb", bufs=2) as xb_pool,
            tc.tile_pool(name="accp", bufs=2) as acc_pool,
            tc.tile_pool(name="ub", bufs=4) as ub_pool,
            tc.tile_pool(name="ob", bufs=4) as ob_pool,
            tc.tile_pool(name="psum", bufs=7, space="PSUM") as psum_pool,
            tc.tile_pool(name="cpsum", bufs=1, space="PSUM") as cpsum_pool,
        ):
            # ---------------- persistent tiles ----------------
            wt = pp.tile([128, CG, 9, CG, 128], BF16)       # lhsT tiles
            zr = pp.tile([128, CG, RING, WPAD], BF16)       # z0 ring
            sm = pp.tile([128, 160], F32)                   # packed small vectors
            dwt = pp.tile([128, CG, 9], F32)
            ws = pp.tile([128, CG, CG * 128], BF16)         # sum_k W per cgi
            btb = pp.tile([128, CG], BF16)                  # beta as bf16 (c-matmul rhs)

            # sm column map
            SUMS, SQS = 0, 32            # CG*NCH = 32 slots each
            MEAN, SQM, INV, SVEC, SSUM, TV, BETA, NEGV, SD, EPSC, SB, CB = (
                64, 68, 72, 76, 80, 84, 88, 92, 96, 100, 104, 108)

            nc.sync.dma_start(dwt[:], dw_ext[:])
            nc.sync.dma_start(sm[:, SVEC:SVEC + CG], pw_ext[:])   # stage pw in SVEC
            nc.sync.dma_start(sm[:, BETA:BETA + CG], bias_ext[:])  # stage bias in BETA
            nc.sync.dma_start(sm[:, SB:SB + CG], sb_ext[:])
            nc.gpsimd.memset(sm[:, EPSC:EPSC + 1], EPS)

            # ---------------- stats + weight prep ----------------
            WTCH = 9 * CG * 128
            inv_n = 1.0 / float(PIX)
            for cg in range(CG):
                # this group's weight slab first, then its stats chunks, so the
                # scaled weights W'[cg] unblock as early as possible
                nc.sync.dma_start(
                    wt[:, cg, :, :, :].rearrange('p b c d -> p (b c d)'),
                    wt_ext[:, cg * WTCH:(cg + 1) * WTCH])
                wv = wt[:, cg, :, :, :].rearrange('p k a b -> p k (a b)')
                nc.vector.tensor_tensor(ws[:, cg, :], wv[:, 0, :], wv[:, 1, :], ALU.add)
                for k in range(2, 9):
                    nc.vector.tensor_tensor(ws[:, cg, :], ws[:, cg, :], wv[:, k, :],
                                            ALU.add)
                for ch in range(NCH):
                    xat = xa_pool.tile([128, XCH], F32)
                    nc.sync.dma_start(
                        xat[:], x_ext[cg * 128:(cg + 1) * 128, ch * XCH:(ch + 1) * XCH])
                    sl = cg * NCH + ch
                    # sum on DVE (single-src 2x mode), bf16 copy feeds ACT square
                    scvt = scrv_pool.tile([128, XCH], BF16)
                    nc.vector.tensor_scalar(scvt[:], xat[:], 1.0, 0.0, ALU.mult,
                                            ALU.add,
                                            accum_out=sm[:, SUMS + sl:SUMS + sl + 1])
                    scrt = scra_pool.tile([128, XCH], BF16)
                    nc.scalar.activation(scrt[:], scvt[:], AF.Square,
                                         accum_out=sm[:, SQS + sl:SQS + sl + 1])
                # everything per channel-group so W'[cg] unblocks as soon as
                # this group's stats chunks land (TensorE staged start)
                nc.vector.tensor_reduce(sm[:, MEAN + cg:MEAN + cg + 1],
                                        sm[:, SUMS + cg * NCH:SUMS + (cg + 1) * NCH],
                                        mybir.AxisListType.X, ALU.add)
                nc.vector.tensor_reduce(sm[:, SQM + cg:SQM + cg + 1],
                                        sm[:, SQS + cg * NCH:SQS + (cg + 1) * NCH],
                                        mybir.AxisListType.X, ALU.add)
                nc.vector.tensor_scalar(sm[:, MEAN + cg:MEAN + cg + 1],
                                        sm[:, MEAN + cg:MEAN + cg + 1],
                                        inv_n, None, ALU.mult)
                nc.vector.tensor_scalar(sm[:, SQM + cg:SQM + cg + 1],
                                        sm[:, SQM + cg:SQM + cg + 1],
                                        inv_n, None, ALU.mult)
                nc.vector.scalar_tensor_tensor(
                    sm[:, NEGV + cg:NEGV + cg + 1], sm[:, MEAN + cg:MEAN + cg + 1],
                    sm[:, MEAN + cg:MEAN + cg + 1], sm[:, SQM + cg:SQM + cg + 1],
                    ALU.mult, ALU.subtract)
                nc.scalar.activation(sm[:, SD + cg:SD + cg + 1],
                                     sm[:, NEGV + cg:NEGV + cg + 1], AF.Sqrt,
                                     bias=sm[:, EPSC:EPSC + 1], scale=-1.0)
                nc.vector.reciprocal(sm[:, INV + cg:INV + cg + 1],
                                     sm[:, SD + cg:SD + cg + 1])
                # s = pw * inv (pw staged in SVEC)
                nc.vector.scalar_tensor_tensor(
                    sm[:, SVEC + cg:SVEC + cg + 1], sm[:, SVEC + cg:SVEC + cg + 1],
                    1.0, sm[:, INV + cg:INV + cg + 1], ALU.mult, ALU.mult)
                # S = sum_k dw
                nc.vector.tensor_reduce(sm[:, SSUM + cg:SSUM + cg + 1],
                                        dwt[:, cg, :], mybir.AxisListType.X, ALU.add)
                # t = mean * s * S
                nc.vector.scalar_tensor_tensor(
                    sm[:, TV + cg:TV + cg + 1], sm[:, MEAN + cg:MEAN + cg + 1],
                    1.0, sm[:, SVEC + cg:SVEC + cg + 1], ALU.mult, ALU.mult)
                nc.vector.scalar_tensor_tensor(
                    sm[:, TV + cg:TV + cg + 1], sm[:, TV + cg:TV + cg + 1],
                    1.0, sm[:, SSUM + cg:SSUM + cg + 1], ALU.mult, ALU.mult)
                # beta = bias - t   (bias staged in BETA)
                nc.vector.scalar_tensor_tensor(
                    sm[:, BETA + cg:BETA + cg + 1], sm[:, TV + cg:TV + cg + 1],
                    -1.0, sm[:, BETA + cg:BETA + cg + 1], ALU.mult, ALU.add)
                nc.vector.tensor_copy(btb[:, cg:cg + 1], sm[:, BETA + cg:BETA + cg + 1])
                # W' = W * s[ci]  (in-place; WAR on the ws reads above)
                wv = wt[:, cg, :, :, :].rearrange('p k a b -> p (k a b)')
                nc.vector.tensor_scalar(wv, wv, sm[:, SVEC + cg:SVEC + cg + 1],
                                        None, ALU.mult)

            def emit_c_mms():
                # c[co] = sum_cgi ws[cgi]^T @ beta[cgi]; then CB = c + spatial_b.
                # Reserved psum bank: band chains must not wait on this slot
                # (its evict depends on stats; a shared slot would deadlock the
                # staged chains whose evicts read CB).
                cpt = cpsum_pool.tile([128, 512], F32)
                for cgo in range(CG):
                    for cgi in range(CG):
                        nc.tensor.matmul(cpt[:, cgo:cgo + 1],
                                         ws[:, cgi, cgo * 128:(cgo + 1) * 128],
                                         btb[:, cgi:cgi + 1],
                                         start=(cgi == 0), stop=(cgi == CG - 1),
                                         skip_group_check=(cgo != 0 or cgi != 0))
                for cgo in range(CG):
                    nc.scalar.activation(sm[:, CB + cgo:CB + cgo + 1],
                                         cpt[:, cgo:cgo + 1],
                                         AF.Identity,
                                         bias=sm[:, SB + cgo:SB + cgo + 1],
                                         scale=1.0)

            # ---------------- z0 production (chunks of 8 rows) ----------------
            def emit_dw_chunk(c):
                r0 = c * ZCH
                xbt = xb_pool.tile([128, CG, ZCH + 2, WPAD], F32)
                lo, hi = r0 - 1, r0 + ZCH
                dlo, dhi = max(lo, 0), min(hi, H - 1)
                for cg in range(CG):
                    src = x_ext[cg * 128:(cg + 1) * 128, :].rearrange(
                        'p (h w) -> p h w', h=H)
                    nc.sync.dma_start(xbt[:, cg, dlo - lo:dhi - lo + 1, 1:W + 1],
                                      src[:, dlo:dhi + 1, :])
                    if lo < 0:
                        nc.sync.dma_start(xbt[:, cg, 0, 1:W + 1], src[:, 1, :])
                    if hi > H - 1:
                        nc.sync.dma_start(xbt[:, cg, ZCH + 1, 1:W + 1], src[:, H - 2, :])
                nc.vector.tensor_copy(xbt[:, :, :, 0:1], xbt[:, :, :, 2:3])
                nc.vector.tensor_copy(xbt[:, :, :, W + 1:W + 2], xbt[:, :, :, W - 1:W])
                s0 = r0 % RING
                for cg in range(CG):
                    acct = acc_pool.tile([128, ZCH, W], F32)
                    for kh in range(3):
                        for kw in range(3):
                            k = kh * 3 + kw
                            xwin = xbt[:, cg, kh:kh + ZCH, kw:kw + W]
                            gs = dwt[:, cg, k:k + 1]
                            if k == 0:
                                nc.vector.tensor_scalar(acct[:], xwin, gs, None,
                                                        ALU.mult)
                            elif k < 8:
                                nc.vector.scalar_tensor_tensor(
                                    acct[:], xwin, gs, acct[:], ALU.mult, ALU.add)
                            else:
                                nc.vector.scalar_tensor_tensor(
                                    zr[:, cg, s0:s0 + ZCH, 1:W + 1], xwin, gs,
                                    acct[:], ALU.mult, ALU.add)
                    nc.vector.tensor_copy(zr[:, cg, s0:s0 + ZCH, 0:1],
                                          zr[:, cg, s0:s0 + ZCH, 2:3])
                    nc.vector.tensor_copy(zr[:, cg, s0:s0 + ZCH, W + 1:W + 2],
                                          zr[:, cg, s0:s0 + ZCH, W - 1:W])

            def slot_runs(r0, kh):
                slots = [_reflect(r0 - 1 + kh + i) % RING for i in range(BAND)]
                runs = []
                i = 0
                while i < BAND:
                    j = i
                    while j + 1 < BAND and slots[j + 1] == slots[j] + 1:
                        j += 1
                    runs.append((slots[i], i, j - i + 1))
                    i = j + 1
                return runs

            def chain_plans(b):
                r0 = b * BAND
                s0 = r0 % RING
                ordered = [(1, 1)] + [(kh, kw) for kh in range(3)
                                      for kw in range(3) if (kh, kw) != (1, 1)]
                plans = []
                for cgi in range(CG):
                    for kh, kw in ordered:
                        runs = ([(s0, 0, BAND)] if kh == 1 else slot_runs(r0, kh))
                        for (sl, off, ln) in runs:
                            plans.append((kh, kw, cgi, sl, off, ln))
                return plans

            def emit_chain_mms(pt, b, cgo, plans, lo, hi):
                total = len(plans)
                for idx in range(lo, hi):
                    kh, kw, cgi, sl, off, ln = plans[idx]
                    rhs = zr[:, cgi, sl:sl + ln, kw:kw + W]
                    lhsT = wt[:, cgi, kh * 3 + kw, cgo, :]
                    nc.tensor.matmul(pt[:, off * W:(off + ln) * W], lhsT, rhs,
                                     start=(idx == 0), stop=(idx == total - 1),
                                     skip_group_check=(idx != 0))

            def emit_evict(pt, b, cgo):
                r0 = b * BAND
                ut = ub_pool.tile([128, BAND * W], F32)
                nc.scalar.activation(ut[:], pt[:], AF.Identity,
                                     bias=sm[:, CB + cgo:CB + cgo + 1], scale=1.0)
                ot = ob_pool.tile([128, BAND * W], F32)
                nc.vector.scalar_tensor_tensor(ot[:], ut[:], SLOPE, ut[:],
                                               ALU.mult, ALU.max)
                nc.sync.dma_start(
                    out_ext[cgo * 128:(cgo + 1) * 128,
                            r0 * W:(r0 + BAND) * W], ot[:])

            def emit_mm_band(b):
                for cgo in range(CG):
                    pt = psum_pool.tile([128, BAND * W], F32, tag="pt")
                    plans = chain_plans(b)
                    emit_chain_mms(pt, b, cgo, plans, 0, len(plans))
                    emit_evict(pt, b, cgo)

            emit_dw_chunk(0)
            emit_dw_chunk(1)
            # Staged prologue: first 7 chains emitted cgi-level-major so the
            # static TensorE stream matches per-channel-group stats arrival
            # (W'[cgi] unblocks group by group while stats still stream).
            staged = [(0, cgo) for cgo in range(CG)] + [(1, cgo) for cgo in range(3)]
            spt = {}
            spl = {}
            for (b, cgo) in staged:
                spt[(b, cgo)] = psum_pool.tile([128, BAND * W], F32, tag="pt", name=f"spt{b}_{cgo}")
                spl[(b, cgo)] = chain_plans(b)
            for lvl in range(CG):
                for key in staged:
                    pl = spl[key]
                    idxs = [i for i, p in enumerate(pl) if p[2] == lvl]
                    emit_chain_mms(spt[key], key[0], key[1], pl,
                                   idxs[0], idxs[-1] + 1)
            emit_c_mms()
            for (b, cgo) in staged:
                emit_evict(spt[(b, cgo)], b, cgo)
            emit_mm_band_one = (1, 3)
            pt13 = psum_pool.tile([128, BAND * W], F32, tag="pt")
            pl13 = chain_plans(1)
            emit_chain_mms(pt13, 1, 3, pl13, 0, len(pl13))
            emit_evict(pt13, 1, 3)

            emitted_chunk = 1
            for b in range(2, NBANDS):
                need = min((b + 1) // 2 + 1, NZCH - 1)
                while emitted_chunk < need:
                    emitted_chunk += 1
                    emit_dw_chunk(emitted_chunk)
                emit_mm_band(b)

    nc.compile()
    return nc


def _get_nc():
    if "nc" not in _CACHE:
        _CACHE["nc"] = _build()
    return _CACHE["nc"]


def _pack_inputs(x, dw_kernels, pw_kernels, biases, spatial_w, spatial_b):
    """Host-side layout packing (no reference math, just reorder/cast)."""
    w = np.asarray(spatial_w, dtype=np.float32).reshape(CG, 128, CG, 128, 9)
    # dims: (cgo, co, cgi, ci, k) -> (ci, cgi, k, cgo, co)
    wt = np.ascontiguousarray(w.transpose(3, 2, 4, 0, 1)).astype(ml_dtypes.bfloat16)
    wt = wt.reshape(128, CG * 9 * CG * 128)

    in_maps = []
    for b in range(B):
        xb = np.ascontiguousarray(np.asarray(x[b], dtype=np.float32).reshape(C, PIX))
        dwb = np.asarray(dw_kernels[b], dtype=np.float32).reshape(CG, 128, 9)
        dwb = np.ascontiguousarray(dwb.transpose(1, 0, 2))            # [128, CG, 9]
        pwb = np.asarray(pw_kernels[b], dtype=np.float32).reshape(CG, 128).T
        bb = np.asarray(biases[b], dtype=np.float32).reshape(CG, 128).T
        sbb = np.asarray(spatial_b, dtype=np.float32).reshape(CG, 128).T
        in_maps.append({
            "x": xb,
            "wt": wt,
            "dw": np.ascontiguousarray(dwb),
            "pw": np.ascontiguousarray(pwb),
            "bias": np.ascontiguousarray(bb),
            "sb": np.ascontiguousarray(sbb),
        })
    return in_maps


def _run(inputs, trace=False):
    from concourse.bass_utils import run_bass_kernel_spmd
    if trace:
        _install_trace_hook()
    nc = _get_nc()
    in_maps = _pack_inputs(**inputs)
    res = run_bass_kernel_spmd(nc, in_maps, core_ids=list(range(B)), trace=trace)
    out = np.stack([res.results[i]["out"].reshape(C, H, W) for i in range(B)])
    return out, res


def _install_trace_hook():
    import types
    try:
        import antenv.axon_hooks  # noqa
    except ImportError:
        from trn_agent_boot.trn_boot import _ntff_profile_via_ctypes
        hook = _ntff_profile_via_ctypes('/opt/axon/libaxon_pjrt.so')
        mod = types.ModuleType('antenv.axon_hooks')
        mod.get_axon_ntff_profile_hook = lambda: hook
        mod.set_axon_ntff_profile_hook = lambda h: None
        sys.modules['antenv.axon_hooks'] = mod


def kernel(**inputs):
    out, _ = _run(inputs, trace=False)
    return out


# revision 15
# speedup vs baseline: 1.0569x; 1.0569x over previous
"""AdaConv2D Trainium2 kernel: instance-norm + per-sample depthwise-separable
conv + dense 3x3 spatial conv + LeakyReLU, data-parallel over batch on 8 cores.

Per core (one batch sample):
  - z0[c] = sum_k dw[c,k] * x[c, window_k]   (raw depthwise, VectorE, f32 acc;
    starts immediately, independent of instance-norm stats)
  - instance norm + pointwise fold into the matmul side:
        out = sum_ci (W * s[ci])^T @ z0[ci] + c[co],  s = pw * rsqrt(var+eps)
        c[co] = sum_ci sum_k W[co,ci,k] * beta[ci],
        beta = bias - mean * s * sum_k dw
    so the dense 3x3 512->512 conv runs on TensorE as 36 accumulating bf16
    matmuls per (4-row band, output channel group) against scaled weights.
  - stats (mean, E[x^2]) stream on ScalarE (sum) + VectorE (sumsq) during the
    same window the depthwise warm-up runs, so TensorE starts ~100us in.
  - LeakyReLU fused as max(0.01*v, v) on VectorE; bias c+spatial_b on ScalarE.
Weights are host-packed/cast to bf16 in the lhsT layout the TensorE wants.
"""
import sys
import numpy as np

sys.path.insert(0, "/opt/trn_rl_repo")

import ml_dtypes  # noqa: E402

B, C, H, W = 8, 512, 128, 128
CG = C // 128          # 4 channel groups
PIX = H * W
BAND = 4               # output rows per matmul band
NBANDS = H // BAND
ZCH = 8                # z production chunk rows
NZCH = H // ZCH
RING = 40              # z ring rows (multiple of ZCH)
WPAD = W + 2           # width-padded row length
XCH = 2048             # pass-A stats chunk (free-dim cols)
NCH = PIX // XCH       # chunks per channel group
EPS = 1e-5
SLOPE = 0.01

_CACHE = {}


def _reflect(r):
    if r < 0:
        return -r
    if r > H - 1:
        return 2 * (H - 1) - r
    return r


def _build():
    from concourse import bacc, tile, mybir

    AF = mybir.ActivationFunctionType
    ALU = mybir.AluOpType
    F32 = mybir.dt.float32
    BF16 = mybir.dt.bfloat16

    nc = bacc.Bacc(None, target_bir_lowering=False, debug=False)

    x_ext = nc.declare_dram_parameter("x", [C, PIX], F32, isOutput=False)
    wt_ext = nc.declare_dram_parameter("wt", [128, CG * 9 * CG * 128], BF16, isOutput=False)
    dw_ext = nc.declare_dram_parameter("dw", [128, CG, 9], F32, isOutput=False)
    pw_ext = nc.declare_dram_parameter("pw", [128, CG], F32, isOutput=False)
    bias_ext = nc.declare_dram_parameter("bias", [128, CG], F32, isOutput=False)
    sb_ext = nc.declare_dram_parameter("sb", [128, CG], F32, isOutput=False)
    out_ext = nc.declare_dram_parameter("out", [C, PIX], F32, isOutput=True)

    with tile.TileContext(nc) as tc:
        with (
            tc.tile_pool(name="persist", bufs=1) as pp,
            tc.tile_pool(name="xa", bufs=3) as xa_pool,
            tc.tile_pool(name="scra", bufs=2) as scra_pool,
            tc.tile_pool(name="scrv", bufs=2) as scrv_pool,
            tc.tile_pool(name="xb", bufs=2) as xb_pool,
            tc.tile_pool(name="accp", bufs=2) as acc_pool,
            tc.tile_pool(name="ub", bufs=4) as ub_pool,
            tc.tile_pool(name="ob", bufs=4) as ob_pool,
            tc.tile_pool(name="psum", bufs=7, space="PSUM") as psum_pool,
            tc.tile_pool(name="cpsum", bufs=1, space="PSUM") as cpsum_pool,
        ):
            # ---------------- persistent tiles ----------------
            wt = pp.tile([128, CG, 9, CG, 128], BF16)       # lhsT tiles
            zr = pp.tile([128, CG, RING, WPAD], BF16)       # z0 ring
            sm = pp.tile([128, 160], F32)                   # packed small vectors
            dwt = pp.tile([128, CG, 9], F32)
            ws = pp.tile([128, CG, CG * 128], BF16)         # sum_k W per cgi
            btb = pp.tile([128, CG], BF16)                  # beta as bf16 (c-matmul rhs)

            # sm column map
            SUMS, SQS = 0, 32            # CG*NCH = 32 slots each
            MEAN, SQM, INV, SVEC, SSUM, TV, BETA, NEGV, SD, EPSC, SB, CB = (
                64, 68, 72, 76, 80, 84, 88, 92, 96, 100, 104, 108)

            nc.sync.dma_start(dwt[:], dw_ext[:])
            nc.sync.dma_start(sm[:, SVEC:SVEC + CG], pw_ext[:])   # stage pw in SVEC
            nc.sync.dma_start(sm[:, BETA:BETA + CG], bias_ext[:])  # stage bias in BETA
            nc.sync.dma_start(sm[:, SB:SB + CG], sb_ext[:])
            nc.gpsimd.memset(sm[:, EPSC:EPSC + 1], EPS)

            # ---------------- stats + weight prep ----------------
            WTCH = 9 * CG * 128
            inv_n = 1.0 / float(PIX)
            for cg in range(CG):
                # this group's weight slab first, then its stats chunks, so the
                # scaled weights W'[cg] unblock as early as possible
                nc.sync.dma_start(
                    wt[:, cg, :, :, :].rearrange('p b c d -> p (b c d)'),
                    wt_ext[:, cg * WTCH:(cg + 1) * WTCH])
                wv = wt[:, cg, :, :, :].rearrange('p k a b -> p k (a b)')
                nc.vector.tensor_tensor(ws[:, cg, :], wv[:, 0, :], wv[:, 1, :], ALU.add)
                for k in range(2, 9):
                    nc.vector.tensor_tensor(ws[:, cg, :], ws[:, cg, :], wv[:, k, :],
                                            ALU.add)
                for ch in range(NCH):
                    xat = xa_pool.tile([128, XCH], F32)
                    nc.sync.dma_start(
                        xat[:], x_ext[cg * 128:(cg + 1) * 128, ch * XCH:(ch + 1) * XCH])
                    sl = cg * NCH + ch
                    # sum on DVE (single-src 2x mode), bf16 copy feeds ACT square
                    scvt = scrv_pool.tile([128, XCH], BF16)
                    nc.vector.tensor_scalar(scvt[:], xat[:], 1.0, 0.0, ALU.mult,
                                            ALU.add,
                                            accum_out=sm[:, SUMS + sl:SUMS + sl + 1])
                    scrt = scra_pool.tile([128, XCH], BF16)
                    nc.scalar.activation(scrt[:], scvt[:], AF.Square,
                                         accum_out=sm[:, SQS + sl:SQS + sl + 1])
                # everything per channel-group so W'[cg] unblocks as soon as
                # this group's stats chunks land (TensorE staged start)
                nc.vector.tensor_reduce(sm[:, MEAN + cg:MEAN + cg + 1],
                                        sm[:, SUMS + cg * NCH:SUMS + (cg + 1) * NCH],
                                        mybir.AxisListType.X, ALU.add)
                nc.vector.tensor_reduce(sm[:, SQM + cg:SQM + cg + 1],
                                        sm[:, SQS + cg * NCH:SQS + (cg + 1) * NCH],
                                        mybir.AxisListType.X, ALU.add)
                nc.vector.tensor_scalar(sm[:, MEAN + cg:MEAN + cg + 1],
                                        sm[:, MEAN + cg:MEAN + cg + 1],
                                        inv_n, None, ALU.mult)
                nc.vector.tensor_scalar(sm[:, SQM + cg:SQM + cg + 1],
                                        sm[:, SQM + cg:SQM + cg + 1],
                                        inv_n, None, ALU.mult)
                nc.vector.scalar_tensor_tensor(
                    sm[:, NEGV + cg:NEGV + cg + 1], sm[:, MEAN + cg:MEAN + cg + 1],
                    sm[:, MEAN + cg:MEAN + cg + 1], sm[:, SQM + cg:SQM + cg + 1],
                    ALU.mult, ALU.subtract)
                nc.scalar.activation(sm[:, SD + cg:SD + cg + 1],
                                     sm[:, NEGV + cg:NEGV + cg + 1], AF.Sqrt,
                                     bias=sm[:, EPSC:EPSC + 1], scale=-1.0)
                nc.vector.reciprocal(sm[:, INV + cg:INV + cg + 1],
                                     sm[:, SD + cg:SD + cg + 1])
                # s = pw * inv (pw staged in SVEC)
                nc.vector.scalar_tensor_tensor(
                    sm[:, SVEC + cg:SVEC + cg + 1], sm[:, SVEC + cg:SVEC + cg + 1],
                    1.0, sm[:, INV + cg:INV + cg + 1], ALU.mult, ALU.mult)
                # S = sum_k dw
                nc.vector.tensor_reduce(sm[:, SSUM + cg:SSUM + cg + 1],
                                        dwt[:, cg, :], mybir.AxisListType.X, ALU.add)
                # t = mean * s * S
                nc.vector.scalar_tensor_tensor(
                    sm[:, TV + cg:TV + cg + 1], sm[:, MEAN + cg:MEAN + cg + 1],
                    1.0, sm[:, SVEC + cg:SVEC + cg + 1], ALU.mult, ALU.mult)
                nc.vector.scalar_tensor_tensor(
                    sm[:, TV + cg:TV + cg + 1], sm[:, TV + cg:TV + cg + 1],
                    1.0, sm[:, SSUM + cg:SSUM + cg + 1], ALU.mult, ALU.mult)
                # beta = bias - t   (bias staged in BETA)
                nc.vector.scalar_tensor_tensor(
                    sm[:, BETA + cg:BETA + cg + 1], sm[:, TV + cg:TV + cg + 1],
                    -1.0, sm[:, BETA + cg:BETA + cg + 1], ALU.mult, ALU.add)
                nc.vector.tensor_copy(btb[:, cg:cg + 1], sm[:, BETA + cg:BETA + cg + 1])
                # W' = W * s[ci]  (in-place; WAR on the ws reads above)
                wv = wt[:, cg, :, :, :].rearrange('p k a b -> p (k a b)')
                nc.vector.tensor_scalar(wv, wv, sm[:, SVEC + cg:SVEC + cg + 1],
                                        None, ALU.mult)

            def emit_c_mms():
                # c[co] = sum_cgi ws[cgi]^T @ beta[cgi]; then CB = c + spatial_b.
                # Reserved psum bank: band chains must not wait on this slot
                # (its evict depends on stats; a shared slot would deadlock the
                # staged chains whose evicts read CB).
                cpt = cpsum_pool.tile([128, 512], F32)
                for cgo in range(CG):
                    for cgi in range(CG):
                        nc.tensor.matmul(cpt[:, cgo:cgo + 1],
                                         ws[:, cgi, cgo * 128:(cgo + 1) * 128],
                                         btb[:, cgi:cgi + 1],
                                         start=(cgi == 0), stop=(cgi == CG - 1),
                                         skip_group_check=(cgo != 0 or cgi != 0))
                for cgo in range(CG):
                    nc.scalar.activation(sm[:, CB + cgo:CB + cgo + 1],
                                         cpt[:, cgo:cgo + 1],
                                         AF.Identity,
                                         bias=sm[:, SB + cgo:SB + cgo + 1],
                                         scale=1.0)

            # ---------------- z0 production (chunks of 8 rows) ----------------
            def emit_dw_chunk(c):
                r0 = c * ZCH
                xbt = xb_pool.tile([128, CG, ZCH + 2, WPAD], F32)
                lo, hi = r0 - 1, r0 + ZCH
                dlo, dhi = max(lo, 0), min(hi, H - 1)
                for cg in range(CG):
                    src = x_ext[cg * 128:(cg + 1) * 128, :].rearrange(
                        'p (h w) -> p h w', h=H)
                    nc.sync.dma_start(xbt[:, cg, dlo - lo:dhi - lo + 1, 1:W + 1],
                                      src[:, dlo:dhi + 1, :])
                    if lo < 0:
                        nc.sync.dma_start(xbt[:, cg, 0, 1:W + 1], src[:, 1, :])
                    if hi > H - 1:
                        nc.sync.dma_start(xbt[:, cg, ZCH + 1, 1:W + 1], src[:, H - 2, :])
                nc.vector.tensor_copy(xbt[:, :, :, 0:1], xbt[:, :, :, 2:3])
                nc.vector.tensor_copy(xbt[:, :, :, W + 1:W + 2], xbt[:, :, :, W - 1:W])
                s0 = r0 % RING
                for cg in range(CG):
                    acct = acc_pool.tile([128, ZCH, W], F32)
                    for kh in range(3):
                        for kw in range(3):
                            k = kh * 3 + kw
                            xwin = xbt[:, cg, kh:kh + ZCH, kw:kw + W]
                            gs = dwt[:, cg, k:k + 1]
                            if k == 0:
                                nc.vector.tensor_scalar(acct[:], xwin, gs, None,
                                                        ALU.mult)
                            elif k < 8:
                                nc.vector.scalar_tensor_tensor(
                                    acct[:], xwin, gs, acct[:], ALU.mult, ALU.add)
                            else:
                                nc.vector.scalar_tensor_tensor(
                                    zr[:, cg, s0:s0 + ZCH, 1:W + 1], xwin, gs,
                                    acct[:], ALU.mult, ALU.add)
                    nc.vector.tensor_copy(zr[:, cg, s0:s0 + ZCH, 0:1],
                                          zr[:, cg, s0:s0 + ZCH, 2:3])
                    nc.vector.tensor_copy(zr[:, cg, s0:s0 + ZCH, W + 1:W + 2],
                                          zr[:, cg, s0:s0 + ZCH, W - 1:W])

            def slot_runs(r0, kh):
                slots = [_reflect(r0 - 1 + kh + i) % RING for i in range(BAND)]
                runs = []
                i = 0
                while i < BAND:
                    j = i
                    while j + 1 < BAND and slots[j + 1] == slots[j] + 1:
                        j += 1
                    runs.append((slots[i], i, j - i + 1))
                    i = j + 1
                return runs

            def chain_plans(b):
                r0 = b * BAND
                s0 = r0 % RING
                ordered = [(1, 1)] + [(kh, kw) for kh in range(3)
                                      for kw in range(3) if (kh, kw) != (1, 1)]
                plans = []
                for cgi in range(CG):
                    for kh, kw in ordered:
                        runs = ([(s0, 0, BAND)] if kh == 1 else slot_runs(r0, kh))
                        for (sl, off, ln) in runs:
                            plans.append((kh, kw, cgi, sl, off, ln))
                return plans

            def emit_chain_mms(pt, b, cgo, plans, lo, hi):
                total = len(plans)
                for idx in range(lo, hi):
                    kh, kw, cgi, sl, off, ln = plans[idx]
                    rhs = zr[:, cgi, sl:sl + ln, kw:kw + W]
                    lhsT = wt[:, cgi, kh * 3 + kw, cgo, :]
                    nc.tensor.matmul(pt[:, off * W:(off + ln) * W], lhsT, rhs,
                                     start=(idx == 0), stop=(idx == total - 1),
                                     skip_group_check=(idx != 0))

            def emit_evict(pt, b, cgo):
                r0 = b * BAND
                ut = ub_pool.tile([128, BAND * W], F32)
                nc.scalar.activation(ut[:], pt[:], AF.Identity,
                                     bias=sm[:, CB + cgo:CB + cgo + 1], scale=1.0)
                ot = ob_pool.tile([128, BAND * W], F32)
                nc.vector.scalar_tensor_tensor(ot[:], ut[:], SLOPE, ut[:],
                                               ALU.mult, ALU.max)
                nc.sync.dma_start(
                    out_ext[cgo * 128:(cgo + 1) * 128,
                            r0 * W:(r0 + BAND) * W], ot[:])

            def emit_mm_band(b):
                for cgo in range(CG):
                    pt = psum_pool.tile([128, BAND * W], F32, tag="pt")
                    plans = chain_plans(b)
                    emit_chain_mms(pt, b, cgo, plans, 0, len(plans))
                    emit_evict(pt, b, cgo)

            emit_dw_chunk(0)
            emit_dw_chunk(1)
            emit_c_mms()
            emitted_chunk = 1
            for b in range(NBANDS):
                need = min((b + 1) // 2 + 1, NZCH - 1)
                while emitted_chunk < need:
                    emitted_chunk += 1
                    emit_dw_chunk(emitted_chunk)
                emit_mm_band(b)

    nc.compile()
    return nc


def _get_nc():
    if "nc" not in _CACHE:
        _CACHE["nc"] = _build()
    return _CACHE["nc"]


def _pack_inputs(x, dw_kernels, pw_kernels, biases, spatial_w, spatial_b):
    """Host-side layout packing (no reference math, just reorder/cast)."""
    w = np.asarray(spatial_w, dtype=np.float32).reshape(CG, 128, CG, 128, 9)
    # dims: (cgo, co, cgi, ci, k) -> (ci, cgi, k, cgo, co)
    wt = np.ascontiguousarray(w.transpose(3, 2, 4, 0, 1)).astype(ml_dtypes.bfloat16)
    wt = wt.reshape(128, CG * 9 * CG * 128)

    in_maps = []
    for b in range(B):
        xb = np.ascontiguousarray(np.asarray(x[b], dtype=np.float32).reshape(C, PIX))
        dwb = np.asarray(dw_kernels[b], dtype=np.float32).reshape(CG, 128, 9)
        dwb = np.ascontiguousarray(dwb.transpose(1, 0, 2))            # [128, CG, 9]
        pwb = np.asarray(pw_kernels[b], dtype=np.float32).reshape(CG, 128).T
        bb = np.asarray(biases[b], dtype=np.float32).reshape(CG, 128).T
        sbb = np.asarray(spatial_b, dtype=np.float32).reshape(CG, 128).T
        in_maps.append({
            "x": xb,
            "wt": wt,
            "dw": np.ascontiguousarray(dwb),
            "pw": np.ascontiguousarray(pwb),
            "bias": np.ascontiguousarray(bb),
            "sb": np.ascontiguousarray(sbb),
        })
    return in_maps


def _run(inputs, trace=False):
    from concourse.bass_utils import run_bass_kernel_spmd
    if trace:
        _install_trace_hook()
    nc = _get_nc()
    in_maps = _pack_inputs(**inputs)
    res = run_bass_kernel_spmd(nc, in_maps, core_ids=list(range(B)), trace=trace)
    out = np.stack([res.results[i]["out"].reshape(C, H, W) for i in range(B)])
    return out, res


def _install_trace_hook():
    import types
    try:
        import antenv.axon_hooks  # noqa
    except ImportError:
        from trn_agent_boot.trn_boot import _ntff_profile_via_ctypes
        hook = _ntff_profile_via_ctypes('/opt/axon/libaxon_pjrt.so')
        mod = types.ModuleType('antenv.axon_hooks')
        mod.get_axon_ntff_profile_hook = lambda: hook
        mod.set_axon_ntff_profile_hook = lambda h: None
        sys.modules['antenv.axon_hooks'] = mod


def kernel(**inputs):
    out, _ = _run(inputs, trace=False)
    return out


# revision 19
# speedup vs baseline: 1.0642x; 1.0069x over previous
"""AdaConv2D Trainium2 kernel: instance-norm + per-sample depthwise-separable
conv + dense 3x3 spatial conv + LeakyReLU, data-parallel over batch on 8 cores.

Per core (one batch sample):
  - z0[c] = sum_k dw[c,k] * x[c, window_k]   (raw depthwise, VectorE, f32 acc;
    starts immediately, independent of instance-norm stats)
  - instance norm + pointwise fold into the matmul side:
        out = sum_ci (W * s[ci])^T @ z0[ci] + c[co],  s = pw * rsqrt(var+eps)
        c[co] = sum_ci sum_k W[co,ci,k] * beta[ci],
        beta = bias - mean * s * sum_k dw
    so the dense 3x3 512->512 conv runs on TensorE as 36 accumulating bf16
    matmuls per (4-row band, output channel group) against scaled weights.
  - stats (mean, E[x^2]) stream on ScalarE (sum) + VectorE (sumsq) during the
    same window the depthwise warm-up runs, so TensorE starts ~100us in.
  - LeakyReLU fused as max(0.01*v, v) on VectorE; bias c+spatial_b on ScalarE.
Weights are host-packed/cast to bf16 in the lhsT layout the TensorE wants.
"""
import sys
import numpy as np

sys.path.insert(0, "/opt/trn_rl_repo")

import ml_dtypes  # noqa: E402

B, C, H, W = 8, 512, 128, 128
CG = C // 128          # 4 channel groups
PIX = H * W
BAND = 4               # output rows per matmul band
NBANDS = H // BAND
ZCH = 8                # z production chunk rows
NZCH = H // ZCH
RING = 40              # z ring rows (multiple of ZCH)
WPAD = W + 2           # width-padded row length
XCH = 2048             # pass-A stats chunk (free-dim cols)
NCH = PIX // XCH       # chunks per channel group
EPS = 1e-5
SLOPE = 0.01

_CACHE = {}


def _reflect(r):
    if r < 0:
        return -r
    if r > H - 1:
        return 2 * (H - 1) - r
    return r


def _build():
    from concourse import bacc, tile, mybir

    AF = mybir.ActivationFunctionType
    ALU = mybir.AluOpType
    F32 = mybir.dt.float32
    BF16 = mybir.dt.bfloat16

    nc = bacc.Bacc(None, target_bir_lowering=False, debug=False)

    x_ext = nc.declare_dram_parameter("x", [C, PIX], BF16, isOutput=False)
    wt_ext = nc.declare_dram_parameter("wt", [128, CG * 9 * CG * 128], BF16, isOutput=False)
    dw_ext = nc.declare_dram_parameter("dw", [128, CG, 9], F32, isOutput=False)
    pw_ext = nc.declare_dram_parameter("pw", [128, CG], F32, isOutput=False)
    bias_ext = nc.declare_dram_parameter("bias", [128, CG], F32, isOutput=False)
    sb_ext = nc.declare_dram_parameter("sb", [128, CG], F32, isOutput=False)
    out_ext = nc.declare_dram_parameter("out", [C, PIX], F32, isOutput=True)

    with tile.TileContext(nc) as tc:
        with (
            tc.tile_pool(name="persist", bufs=1) as pp,
            tc.tile_pool(name="xa", bufs=3) as xa_pool,
            tc.tile_pool(name="scra", bufs=2) as scra_pool,
            tc.tile_pool(name="scrv", bufs=2) as scrv_pool,
            tc.tile_pool(name="xb", bufs=2) as xb_pool,
            tc.tile_pool(name="accp", bufs=2) as acc_pool,
            tc.tile_pool(name="ub", bufs=4) as ub_pool,
            tc.tile_pool(name="ob", bufs=4) as ob_pool,
            tc.tile_pool(name="psum", bufs=7, space="PSUM") as psum_pool,
            tc.tile_pool(name="cpsum", bufs=1, space="PSUM") as cpsum_pool,
        ):
            # ---------------- persistent tiles ----------------
            wt = pp.tile([128, CG, 9, CG, 128], BF16)       # lhsT tiles
            zr = pp.tile([128, CG, RING, WPAD], BF16)       # z0 ring
            sm = pp.tile([128, 160], F32)                   # packed small vectors
            dwt = pp.tile([128, CG, 9], F32)
            ws = pp.tile([128, CG, CG * 128], BF16)         # sum_k W per cgi
            btb = pp.tile([128, CG], BF16)                  # beta as bf16 (c-matmul rhs)

            # sm column map
            SUMS, SQS = 0, 32            # CG*NCH = 32 slots each
            MEAN, SQM, INV, SVEC, SSUM, TV, BETA, NEGV, SD, EPSC, SB, CB = (
                64, 68, 72, 76, 80, 84, 88, 92, 96, 100, 104, 108)

            nc.sync.dma_start(dwt[:], dw_ext[:])
            nc.sync.dma_start(sm[:, SVEC:SVEC + CG], pw_ext[:])   # stage pw in SVEC
            nc.sync.dma_start(sm[:, BETA:BETA + CG], bias_ext[:])  # stage bias in BETA
            nc.sync.dma_start(sm[:, SB:SB + CG], sb_ext[:])
            nc.gpsimd.memset(sm[:, EPSC:EPSC + 1], EPS)

            # ---------------- stats + weight prep ----------------
            WTCH = 9 * CG * 128
            inv_n = 1.0 / float(PIX)
            for cg in range(CG):
                # this group's weight slab first, then its stats chunks, so the
                # scaled weights W'[cg] unblock as early as possible
                nc.sync.dma_start(
                    wt[:, cg, :, :, :].rearrange('p b c d -> p (b c d)'),
                    wt_ext[:, cg * WTCH:(cg + 1) * WTCH])
                for ch in range(NCH):
                    xat = xa_pool.tile([128, XCH], BF16)
                    nc.sync.dma_start(
                        xat[:], x_ext[cg * 128:(cg + 1) * 128, ch * XCH:(ch + 1) * XCH])
                    sl = cg * NCH + ch
                    # sum on DVE (single-src bf16 4x mode)
                    scvt = scrv_pool.tile([128, XCH], BF16)
                    nc.vector.tensor_scalar(scvt[:], xat[:], 1.0, 0.0, ALU.mult,
                                            ALU.add,
                                            accum_out=sm[:, SUMS + sl:SUMS + sl + 1])
                    # sumsq alternates ScalarE / VectorE so DMA stays the pacer
                    if ch % 2 == 0:
                        scrt = scra_pool.tile([128, XCH], BF16)
                        nc.scalar.activation(scrt[:], xat[:], AF.Square,
                                             accum_out=sm[:, SQS + sl:SQS + sl + 1])
                    else:
                        scqt = scrv_pool.tile([128, XCH], BF16, tag="scvt")
                        nc.vector.scalar_tensor_tensor(
                            scqt[:], xat[:], 1.0, xat[:], ALU.mult, ALU.mult,
                            accum_out=sm[:, SQS + sl:SQS + sl + 1])
                # ws[cg] = sum_k W (after the chunks: keeps chunk 0's stats ops
                # at the head of the VectorE stream instead of behind the
                # weight-slab DMA wait)
                wv = wt[:, cg, :, :, :].rearrange('p k a b -> p k (a b)')
                nc.vector.tensor_tensor(ws[:, cg, :], wv[:, 0, :], wv[:, 1, :], ALU.add)
                for k in range(2, 9):
                    nc.vector.tensor_tensor(ws[:, cg, :], ws[:, cg, :], wv[:, k, :],
                                            ALU.add)
                # everything per channel-group so W'[cg] unblocks as soon as
                # this group's stats chunks land (TensorE staged start)
                nc.vector.tensor_reduce(sm[:, MEAN + cg:MEAN + cg + 1],
                                        sm[:, SUMS + cg * NCH:SUMS + (cg + 1) * NCH],
                                        mybir.AxisListType.X, ALU.add)
                nc.vector.tensor_reduce(sm[:, SQM + cg:SQM + cg + 1],
                                        sm[:, SQS + cg * NCH:SQS + (cg + 1) * NCH],
                                        mybir.AxisListType.X, ALU.add)
                nc.vector.tensor_scalar(sm[:, MEAN + cg:MEAN + cg + 1],
                                        sm[:, MEAN + cg:MEAN + cg + 1],
                                        inv_n, None, ALU.mult)
                nc.vector.tensor_scalar(sm[:, SQM + cg:SQM + cg + 1],
                                        sm[:, SQM + cg:SQM + cg + 1],
                                        inv_n, None, ALU.mult)
                nc.vector.scalar_tensor_tensor(
                    sm[:, NEGV + cg:NEGV + cg + 1], sm[:, MEAN + cg:MEAN + cg + 1],
                    sm[:, MEAN + cg:MEAN + cg + 1], sm[:, SQM + cg:SQM + cg + 1],
                    ALU.mult, ALU.subtract)
                nc.scalar.activation(sm[:, SD + cg:SD + cg + 1],
                                     sm[:, NEGV + cg:NEGV + cg + 1], AF.Sqrt,
                                     bias=sm[:, EPSC:EPSC + 1], scale=-1.0)
                nc.vector.reciprocal(sm[:, INV + cg:INV + cg + 1],
                                     sm[:, SD + cg:SD + cg + 1])
                # s = pw * inv (pw staged in SVEC)
                nc.vector.scalar_tensor_tensor(
                    sm[:, SVEC + cg:SVEC + cg + 1], sm[:, SVEC + cg:SVEC + cg + 1],
                    1.0, sm[:, INV + cg:INV + cg + 1], ALU.mult, ALU.mult)
                # S = sum_k dw
                nc.vector.tensor_reduce(sm[:, SSUM + cg:SSUM + cg + 1],
                                        dwt[:, cg, :], mybir.AxisListType.X, ALU.add)
                # t = mean * s * S
                nc.vector.scalar_tensor_tensor(
                    sm[:, TV + cg:TV + cg + 1], sm[:, MEAN + cg:MEAN + cg + 1],
                    1.0, sm[:, SVEC + cg:SVEC + cg + 1], ALU.mult, ALU.mult)
                nc.vector.scalar_tensor_tensor(
                    sm[:, TV + cg:TV + cg + 1], sm[:, TV + cg:TV + cg + 1],
                    1.0, sm[:, SSUM + cg:SSUM + cg + 1], ALU.mult, ALU.mult)
                # beta = bias - t   (bias staged in BETA)
                nc.vector.scalar_tensor_tensor(
                    sm[:, BETA + cg:BETA + cg + 1], sm[:, TV + cg:TV + cg + 1],
                    -1.0, sm[:, BETA + cg:BETA + cg + 1], ALU.mult, ALU.add)
                nc.vector.tensor_copy(btb[:, cg:cg + 1], sm[:, BETA + cg:BETA + cg + 1])
                # W' = W * s[ci]  (in-place; WAR on the ws reads above)
                wv = wt[:, cg, :, :, :].rearrange('p k a b -> p (k a b)')
                nc.vector.tensor_scalar(wv, wv, sm[:, SVEC + cg:SVEC + cg + 1],
                                        None, ALU.mult)

            def emit_c_mms():
                # c[co] = sum_cgi ws[cgi]^T @ beta[cgi]; then CB = c + spatial_b.
                # Reserved psum bank: band chains must not wait on this slot
                # (its evict depends on stats; a shared slot would deadlock the
                # staged chains whose evicts read CB).
                cpt = cpsum_pool.tile([128, 512], F32)
                for cgo in range(CG):
                    for cgi in range(CG):
                        nc.tensor.matmul(cpt[:, cgo:cgo + 1],
                                         ws[:, cgi, cgo * 128:(cgo + 1) * 128],
                                         btb[:, cgi:cgi + 1],
                                         start=(cgi == 0), stop=(cgi == CG - 1),
                                         skip_group_check=(cgo != 0 or cgi != 0))
                for cgo in range(CG):
                    nc.scalar.activation(sm[:, CB + cgo:CB + cgo + 1],
                                         cpt[:, cgo:cgo + 1],
                                         AF.Identity,
                                         bias=sm[:, SB + cgo:SB + cgo + 1],
                                         scale=1.0)

            # ---------------- z0 production (chunks of 8 rows) ----------------
            def emit_dw_chunk(c):
                r0 = c * ZCH
                xbt = xb_pool.tile([128, CG, ZCH + 2, WPAD], BF16)
                lo, hi = r0 - 1, r0 + ZCH
                dlo, dhi = max(lo, 0), min(hi, H - 1)
                for cg in range(CG):
                    src = x_ext[cg * 128:(cg + 1) * 128, :].rearrange(
                        'p (h w) -> p h w', h=H)
                    nc.sync.dma_start(xbt[:, cg, dlo - lo:dhi - lo + 1, 1:W + 1],
                                      src[:, dlo:dhi + 1, :])
                    if lo < 0:
                        nc.sync.dma_start(xbt[:, cg, 0, 1:W + 1], src[:, 1, :])
                    if hi > H - 1:
                        nc.sync.dma_start(xbt[:, cg, ZCH + 1, 1:W + 1], src[:, H - 2, :])
                nc.vector.tensor_copy(xbt[:, :, :, 0:1], xbt[:, :, :, 2:3])
                nc.vector.tensor_copy(xbt[:, :, :, W + 1:W + 2], xbt[:, :, :, W - 1:W])
                s0 = r0 % RING
                for cg in range(CG):
                    acct = acc_pool.tile([128, ZCH, W], F32)
                    for kh in range(3):
                        for kw in range(3):
                            k = kh * 3 + kw
                            xwin = xbt[:, cg, kh:kh + ZCH, kw:kw + W]
                            gs = dwt[:, cg, k:k + 1]
                            if k == 0:
                                nc.vector.tensor_scalar(acct[:], xwin, gs, None,
                                                        ALU.mult)
                            elif k < 8:
                                nc.vector.scalar_tensor_tensor(
                                    acct[:], xwin, gs, acct[:], ALU.mult, ALU.add)
                            else:
                                nc.vector.scalar_tensor_tensor(
                                    zr[:, cg, s0:s0 + ZCH, 1:W + 1], xwin, gs,
                                    acct[:], ALU.mult, ALU.add)
                    nc.vector.tensor_copy(zr[:, cg, s0:s0 + ZCH, 0:1],
                                          zr[:, cg, s0:s0 + ZCH, 2:3])
                    nc.vector.tensor_copy(zr[:, cg, s0:s0 + ZCH, W + 1:W + 2],
                                          zr[:, cg, s0:s0 + ZCH, W - 1:W])

            def slot_runs(r0, kh):
                slots = [_reflect(r0 - 1 + kh + i) % RING for i in range(BAND)]
                runs = []
                i = 0
                while i < BAND:
                    j = i
                    while j + 1 < BAND and slots[j + 1] == slots[j] + 1:
                        j += 1
                    runs.append((slots[i], i, j - i + 1))
                    i = j + 1
                return runs

            def chain_plans(b):
                r0 = b * BAND
                s0 = r0 % RING
                ordered = [(1, 1)] + [(kh, kw) for kh in range(3)
                                      for kw in range(3) if (kh, kw) != (1, 1)]
                plans = []
                for cgi in range(CG):
                    for kh, kw in ordered:
                        runs = ([(s0, 0, BAND)] if kh == 1 else slot_runs(r0, kh))
                        for (sl, off, ln) in runs:
                            plans.append((kh, kw, cgi, sl, off, ln))
                return plans

            def emit_chain_mms(pt, b, cgo, plans, lo, hi):
                total = len(plans)
                for idx in range(lo, hi):
                    kh, kw, cgi, sl, off, ln = plans[idx]
                    rhs = zr[:, cgi, sl:sl + ln, kw:kw + W]
                    lhsT = wt[:, cgi, kh * 3 + kw, cgo, :]
                    nc.tensor.matmul(pt[:, off * W:(off + ln) * W], lhsT, rhs,
                                     start=(idx == 0), stop=(idx == total - 1),
                                     skip_group_check=(idx != 0))

            def emit_evict(pt, b, cgo):
                r0 = b * BAND
                ut = ub_pool.tile([128, BAND * W], F32)
                nc.scalar.activation(ut[:], pt[:], AF.Identity,
                                     bias=sm[:, CB + cgo:CB + cgo + 1], scale=1.0)
                ot = ob_pool.tile([128, BAND * W], F32)
                nc.vector.scalar_tensor_tensor(ot[:], ut[:], SLOPE, ut[:],
                                               ALU.mult, ALU.max)
                nc.sync.dma_start(
                    out_ext[cgo * 128:(cgo + 1) * 128,
                            r0 * W:(r0 + BAND) * W], ot[:])

            def emit_mm_band(b):
                for cgo in range(CG):
                    pt = psum_pool.tile([128, BAND * W], F32, tag="pt")
                    plans = chain_plans(b)
                    emit_chain_mms(pt, b, cgo, plans, 0, len(plans))
                    emit_evict(pt, b, cgo)

            emit_dw_chunk(0)
            emit_dw_chunk(1)
            emit_c_mms()
            emitted_chunk = 1
            for b in range(NBANDS):
                need = min((b + 1) // 2 + 1, NZCH - 1)
                while emitted_chunk < need:
                    emitted_chunk += 1
                    emit_dw_chunk(emitted_chunk)
                emit_mm_band(b)

    nc.compile()
    return nc


def _get_nc():
    if "nc" not in _CACHE:
        _CACHE["nc"] = _build()
    return _CACHE["nc"]


def _pack_inputs(x, dw_kernels, pw_kernels, biases, spatial_w, spatial_b):
    """Host-side layout packing (no reference math, just reorder/cast)."""
    w = np.asarray(spatial_w, dtype=np.float32).reshape(CG, 128, CG, 128, 9)
    # dims: (cgo, co, cgi, ci, k) -> (ci, cgi, k, cgo, co)
    wt = np.ascontiguousarray(w.transpose(3, 2, 4, 0, 1)).astype(ml_dtypes.bfloat16)
    wt = wt.reshape(128, CG * 9 * CG * 128)

    in_maps = []
    for b in range(B):
        xb = np.ascontiguousarray(
            np.asarray(x[b], dtype=np.float32).reshape(C, PIX)).astype(
                ml_dtypes.bfloat16)
        dwb = np.asarray(dw_kernels[b], dtype=np.float32).reshape(CG, 128, 9)
        dwb = np.ascontiguousarray(dwb.transpose(1, 0, 2))            # [128, CG, 9]
        pwb = np.asarray(pw_kernels[b], dtype=np.float32).reshape(CG, 128).T
        bb = np.asarray(biases[b], dtype=np.float32).reshape(CG, 128).T
        sbb = np.asarray(spatial_b, dtype=np.float32).reshape(CG, 128).T
        in_maps.append({
            "x": xb,
            "wt": wt,
            "dw": np.ascontiguousarray(dwb),
            "pw": np.ascontiguousarray(pwb),
            "bias": np.ascontiguousarray(bb),
            "sb": np.ascontiguousarray(sbb),
        })
    return in_maps


def _run(inputs, trace=False):
    from concourse.bass_utils import run_bass_kernel_spmd
    if trace:
        _install_trace_hook()
    nc = _get_nc()
    in_maps = _pack_inputs(**inputs)
    res = run_bass_kernel_spmd(nc, in_maps, core_ids=list(range(B)), trace=trace)
    out = np.stack([res.results[i]["out"].reshape(C, H, W) for i in range(B)])
    return out, res


def _install_trace_hook():
    import types
    try:
        import antenv.axon_hooks  # noqa
    except ImportError:
        from trn_agent_boot.trn_boot import _ntff_profile_via_ctypes
        hook = _ntff_profile_via_ctypes('/opt/axon/libaxon_pjrt.so')
        mod = types.ModuleType('antenv.axon_hooks')
        mod.get_axon_ntff_profile_hook = lambda: hook
        mod.set_axon_ntff_profile_hook = lambda h: None
        sys.modules['antenv.axon_hooks'] = mod


def kernel(**inputs):
    out, _ = _run(inputs, trace=False)
    return out
